# revision 6
# baseline (speedup 1.0000x reference)
"""E8 RHT Embedding kernel for Trainium2 (8 NeuronCores, data-parallel over tokens).

Math (reference): out[t] = SV * H2048( Wscale * (cb1[Qidxs[id_t]] + irs*cb2[Qidxs2[id_t]]) )
with H2048 the unnormalized Sylvester Hadamard transform over 2048 dims,
each Qidxs row holding 256 uint16 codebook indices of 8-dim E8 entries.

Strategy:
  Host (weight-only preprocessing, no token-dependent compute):
    - Fold H8 (inner Kronecker factor) + Wscale/inv_resid_scale into the codebooks,
      pre-dequantize the whole embedding table E[v] (131072 x 2048 fp16), and permute
      columns so the on-chip transposing gather lands data matmul-ready:
        E[v, (2j+r)*128 + i_lo] = (cb1H[Qidxs[v,i]] + cb2H[Qidxs2[v,i]])[j],  i = r*128+i_lo
    - Fold H2 (outer sign) and the SV output scale into 16 moving matrices
      Smov[j,r][k, q*128+p'] = sign(q,r) * SV[(q*128+p')*8+j] * H128[k, p'].
  Device (per core, 2048 tokens):
    - gpsimd dma_gather(transpose=True): fetch token rows of E (4KB each) transposed
      into SBUF X[i_lo, 2j+r, slot]. int16 gather indices only span 32768 rows, so
      tokens are processed sorted by vocab quarter (4 gather calls), padded to 128.
    - TensorE: per 128-slot chunk, per j: PSUM[t, (q,p')] = X_chunk^T-contraction with
      Smov (gathered data is the *stationary* operand), accumulating the two r halves.
      This directly yields token-major output rows - no transpose needed.
    - ScalarE: PSUM (j,q,p') fp32 -> SBUF token rows (q,p',j) fp16.
    - gpsimd indirect scatter: write each token's 4KB row to its original position
      (padding slots go to a dummy trailing row).
"""
import sys
import numpy as np

if "/opt/trn_rl_repo" not in sys.path:
    sys.path.insert(0, "/opt/trn_rl_repo")

VOCAB = 131072
D = 2048
NCORES = 8
TOK_PER_CORE = 2048
QUARTER = 32768

_TRACE = [False]
_LAST_RESULTS = [None]


def _hadamard(n):
    H = np.array([[1.0]], dtype=np.float64)
    while H.shape[0] < n:
        H = np.block([[H, H], [H, -H]])
    return H


def _host_prep_weights(Qidxs, Qidxs2, codebook, codebook2, SV, Wscale, inv_resid_scale):
    H8 = _hadamard(8).astype(np.float32)
    H128 = _hadamard(128).astype(np.float32)
    ws = float(np.asarray(Wscale))
    irs = float(np.asarray(inv_resid_scale))
    cb1H = (codebook.astype(np.float32) @ H8) * ws
    cb2H = (codebook2.astype(np.float32) @ H8) * (ws * irs)

    # column permutation: dst col (2j+r)*128+i_lo takes src (i, j), i=r*128+i_lo
    i_idx = np.arange(256)
    j_idx = np.arange(8)
    dst_col = (2 * j_idx[None, :] + (i_idx >> 7)[:, None]) * 128 + (i_idx & 127)[:, None]
    perm = np.empty(D, dtype=np.int64)
    perm[dst_col.reshape(-1)] = np.arange(D)

    E = np.empty((VOCAB, D), dtype=np.float16)
    CH = 8192
    for v0 in range(0, VOCAB, CH):
        q1 = Qidxs[v0:v0 + CH].astype(np.int32) & 0xFFFF
        q2 = Qidxs2[v0:v0 + CH].astype(np.int32) & 0xFFFF
        G = (cb1H[q1] + cb2H[q2]).reshape(-1, D)
        E[v0:v0 + CH] = G[:, perm].astype(np.float16)

    SVf = SV.astype(np.float32).reshape(2, 128, 8)  # [q, p', j]
    Smv = np.empty((128, 16, 256), dtype=np.float16)  # [k, j*2+r, q*128+p']
    for j in range(8):
        for r in range(2):
            cols = np.empty((128, 256), np.float32)
            for q in range(2):
                sign = -1.0 if (q == 1 and r == 1) else 1.0
                cols[:, q * 128:(q + 1) * 128] = H128 * (sign * SVf[q, :, j])[None, :]
            Smv[:, j * 2 + r, :] = cols.astype(np.float16)
    return E, Smv


def _host_prep_tokens(flat_ids):
    """Per-core quarter-sorted slot bookkeeping. Returns caps plus per-core
    (idx_wrapped, offs, ) arrays."""
    counts = np.zeros((NCORES, 4), dtype=np.int64)
    percore = []
    for c in range(NCORES):
        v = flat_ids[c * TOK_PER_CORE:(c + 1) * TOK_PER_CORE]
        percore.append(v)
        for k in range(4):
            counts[c, k] = int(((v >> 15) == k).sum())
    caps = []
    for k in range(4):
        m = int(counts[:, k].max())
        caps.append(0 if m == 0 else int(np.ceil(m / 128) * 128))
    S = sum(caps)
    nch = S // 128

    idx_all = np.zeros((NCORES, 128, S // 16), dtype=np.int16)
    offs_all = np.full((NCORES, 128, nch), D, dtype=np.int32)  # default -> dummy row D(=2048)
    for c in range(NCORES):
        v = percore[c]
        order = np.argsort(v >> 15, kind="stable")
        col = 0
        slot_rows = np.empty(S, dtype=np.int32)
        slot_orig = np.full(S, -1, dtype=np.int32)
        for k in range(4):
            if caps[k] == 0:
                continue
            sel = order[(v[order] >> 15) == k]
            nk = len(sel)
            rows = np.full(caps[k], k * QUARTER, dtype=np.int32)
            rows[:nk] = v[sel]
            slot_rows[col:col + caps[k]] = rows
            slot_orig[col:col + nk] = sel
            col += caps[k]
        # wrapped idx arrays per 128-slot chunk, replicated across the 8 groups
        # of 16 partitions; chunk ch covers slots [ch*128, ch*128+128), all within
        # one region by construction (caps are multiples of 128).
        region_of = np.repeat(np.arange(4), [caps[k] for k in range(4)])
        for ch in range(nch):
            sl = slot_rows[ch * 128:(ch + 1) * 128]
            k = int(region_of[ch * 128])
            local = (sl - k * QUARTER).astype(np.int16)
            wr = local.reshape(8, 16).T  # [16, 8]
            idx_all[c, :, ch * 8:(ch + 1) * 8] = np.tile(wr, (8, 1))
        for ch in range(nch):
            so = slot_orig[ch * 128:(ch + 1) * 128]
            offs_all[c, :, ch] = np.where(so >= 0, so, D)
    return caps, S, nch, idx_all, offs_all


def _build_program(caps, S, nch):
    import concourse.bacc as bacc
    import concourse.bass as bass
    import concourse.mybir as mybir
    from concourse.bass import IndirectOffsetOnAxis
    from concourse.library_config import mlp
    from contextlib import ExitStack

    regions = [k for k in range(4) if caps[k] > 0]
    # gather calls: per region, groups of up to 512 slots (multiples of 128);
    # chunk ch -> (call index, local t0 within call); call -> (region, slot0, size)
    calls = []
    chunk_map = []
    slot0 = 0
    for k in regions:
        done = 0
        while done < caps[k]:
            csz = min(512, caps[k] - done)
            ci = len(calls)
            calls.append((k, slot0 + done, csz))
            for t0 in range(0, csz, 128):
                chunk_map.append((ci, t0))
            done += csz
        slot0 += caps[k]
    assert len(chunk_map) == nch

    nc = bacc.Bacc("TRN2", debug=True, dynamic_dma_scratch_size=65536)
    E_d = nc.dram_tensor("E", [VOCAB, D], mybir.dt.float16, kind="ExternalInput")
    S_d = nc.dram_tensor("Smv", [128, 16 * 256], mybir.dt.float16, kind="ExternalInput")
    idx_d = nc.dram_tensor("idx", [128, S // 16], mybir.dt.int16, kind="ExternalInput")
    offs_d = nc.dram_tensor("offs", [128, nch], mybir.dt.int32, kind="ExternalInput")
    out_d = nc.dram_tensor("out", [D + 1, D], mybir.dt.float16, kind="ExternalOutput")

    with (
        nc.Block() as block,
        ExitStack() as st,
        nc.semaphore("io") as io,
        nc.semaphore("gsem") as gsem,
        nc.semaphore("mmsem") as mmsem,
        nc.semaphore("evsemA") as evsemA,
        nc.semaphore("evsemB") as evsemB,
        nc.semaphore("scsem") as scsem,
    ):
        smv_sb = st.enter_context(nc.sbuf_tensor("smv", [128, 16, 256], mybir.dt.float16))
        idx_sb = st.enter_context(nc.sbuf_tensor("idxs", [128, S // 16], mybir.dt.int16))
        x_c = [st.enter_context(nc.sbuf_tensor(f"x{ci}", [128, 16, csz], mybir.dt.float16))
               for ci, (k, s0, csz) in enumerate(calls)]
        t2 = [st.enter_context(nc.sbuf_tensor(f"t2_{i}", [128, D], mybir.dt.float16))
              for i in range(3)]
        fo_sb = st.enter_context(nc.sbuf_tensor("fo", [128, nch], mybir.dt.int32))
        ps = [st.enter_context(nc.psum_tensor(f"ps{i}", [128, 8, 2, 128], mybir.dt.float32))
              for i in range(2)]

        @block.sync
        def _(sync):
            sync.dma_start(idx_sb[:, :], idx_d[:, :]).then_inc(io, 16)
            sync.dma_start(smv_sb[:, :, :], S_d[:, :].rearrange("p (a b) -> p a b", a=16)).then_inc(io, 16)
            sync.dma_start(fo_sb[:, :], offs_d[:, :]).then_inc(io, 16)

        def emit_gather(gpsimd, ci):
            k, s0, csz = calls[ci]
            gpsimd.dma_gather(
                x_c[ci][:, :, :],
                E_d[k * QUARTER:(k + 1) * QUARTER, :],
                idx_sb[:, s0 // 16:(s0 + csz) // 16],
                csz, csz, D,
                transpose=True,
            ).then_inc(gsem, 16)

        @block.gpsimd
        def _(gpsimd):
            gpsimd.load_library(mlp)
            gpsimd.wait_ge(io, 16)
            for ci in range(len(calls)):
                emit_gather(gpsimd, ci)
            gpsimd.wait_ge(io, 48)
            for ch in range(nch):
                if ch % 2 == 0:
                    gpsimd.wait_ge(evsemA, ch // 2 + 1)
                else:
                    gpsimd.wait_ge(evsemB, (ch + 1) // 2)
                gpsimd.indirect_dma_start(
                    out_d[:, :],
                    IndirectOffsetOnAxis(ap=fo_sb[:, ch:ch + 1], axis=0),
                    t2[ch % 3][:, :],
                    None,
                ).then_inc(scsem, 16)

        @block.tensor
        def _(tensor):
            import concourse.bass as bass
            tensor.wait_ge(io, 32)
            for ch, (ci, t0) in enumerate(chunk_map):
                tensor.wait_ge(gsem, 16 * (ci + 1))
                if ch >= 2:
                    if ch % 2 == 0:
                        tensor.wait_ge(evsemA, ch // 2)
                    else:
                        tensor.wait_ge(evsemB, (ch - 1) // 2)
                p = ps[ch % 2]
                for j in range(8):
                    for r in range(2):
                        mm = tensor.matmul(
                            p[:, j, :, :],
                            bass.AP(x_c[ci], (2 * j + r) * calls[ci][2] + t0,
                                    [[16 * calls[ci][2], 128], [1, 128]]),
                            smv_sb[:, j * 2 + r, :],
                            start=(r == 0),
                            stop=(r == 1),
                        )
                        if j == 7 and r == 1:
                            mm.then_inc(mmsem)

        # evac enumeration (q, p', j): contiguous 8-elem writes, strided reads
        # (strided *writes* cost ~4.4ns/elem; strided reads are free).
        def evac_aps(ch):
            p = ps[ch % 2]
            out_ap = bass.AP(t2[ch % 3], 0, [[D, 128], [1024, 2], [8, 128], [1, 8]])
            in_ap = bass.AP(p, 0, [[2048, 128], [128, 2], [1, 128], [256, 8]])
            return out_ap, in_ap

        @block.scalar
        def _(scalar):
            import concourse.mybir as mybir
            for ch in range(0, nch, 2):
                scalar.wait_ge(mmsem, ch + 1)
                if ch >= 3:
                    scalar.wait_ge(scsem, 16 * (ch - 2))
                out_ap, in_ap = evac_aps(ch)
                scalar.activation(out_ap, in_ap, mybir.ActivationFunctionType.Copy).then_inc(evsemA)

        @block.vector
        def _(vector):
            for ch in range(1, nch, 2):
                vector.wait_ge(mmsem, ch + 1)
                if ch >= 3:
                    vector.wait_ge(scsem, 16 * (ch - 2))
                out_ap, in_ap = evac_aps(ch)
                vector.tensor_copy(out_ap, in_ap).then_inc(evsemB)

    nc.compile()
    return nc


def kernel(input_ids, Qidxs, Qidxs2, codebook, codebook2, SV, Wscale, inv_resid_scale):
    from concourse.bass_utils import run_bass_kernel_spmd

    input_ids = np.asarray(input_ids)
    E, Smv = _host_prep_weights(np.asarray(Qidxs), np.asarray(Qidxs2),
                                np.asarray(codebook), np.asarray(codebook2),
                                np.asarray(SV), Wscale, inv_resid_scale)
    flat = input_ids.reshape(-1).astype(np.int64)
    caps, S, nch, idx_all, offs_all = _host_prep_tokens(flat)
    nc = _build_program(caps, S, nch)

    smv_flat = Smv.reshape(128, 16 * 256)
    in_maps = []
    for c in range(NCORES):
        in_maps.append({
            "E": E,
            "Smv": smv_flat,
            "idx": idx_all[c],
            "offs": offs_all[c],
        })
    res = run_bass_kernel_spmd(nc, in_maps, core_ids=list(range(NCORES)),
                               trace=_TRACE[0])
    _LAST_RESULTS[0] = res
    out = np.empty((NCORES, TOK_PER_CORE, D), dtype=np.float16)
    for c in range(NCORES):
        out[c] = res.results[c]["out"][:D]
    return out.reshape(input_ids.shape + (D,))


# revision 9
# speedup vs baseline: 1.3946x; 1.3946x over previous
"""E8 RHT Embedding kernel for Trainium2 (8 NeuronCores, data-parallel over tokens).

Math (reference): out[t] = SV * H2048( Wscale * (cb1[Qidxs[id_t]] + irs*cb2[Qidxs2[id_t]]) )
with H2048 the unnormalized Sylvester Hadamard transform over 2048 dims,
each Qidxs row holding 256 uint16 codebook indices of 8-dim E8 entries.

Strategy:
  Host (weight-only preprocessing, no token-dependent compute):
    - Fold H8 (inner Kronecker factor) + Wscale/inv_resid_scale into the codebooks,
      pre-dequantize the whole embedding table E[v] (131072 x 2048 fp16), and permute
      columns so the on-chip transposing gather lands data matmul-ready:
        E[v, (2j+r)*128 + i_lo] = (cb1H[Qidxs[v,i]] + cb2H[Qidxs2[v,i]])[j],  i = r*128+i_lo
    - Fold H2 (outer sign) and the SV output scale into 16 moving matrices
      Smov[j,r][k, q*128+p'] = sign(q,r) * SV[(q*128+p')*8+j] * H128[k, p'].
  Device (per core, 2048 tokens):
    - gpsimd dma_gather(transpose=True): fetch token rows of E (4KB each) transposed
      into SBUF X[i_lo, 2j+r, slot]. int16 gather indices only span 32768 rows, so
      tokens are processed sorted by vocab quarter (4 gather calls), padded to 128.
    - TensorE: per 128-slot chunk, per j: PSUM[t, (q,p')] = X_chunk^T-contraction with
      Smov (gathered data is the *stationary* operand), accumulating the two r halves.
      This directly yields token-major output rows - no transpose needed.
    - ScalarE: PSUM (j,q,p') fp32 -> SBUF token rows (q,p',j) fp16.
    - gpsimd indirect scatter: write each token's 4KB row to its original position
      (padding slots go to a dummy trailing row).
"""
import sys
import numpy as np

if "/opt/trn_rl_repo" not in sys.path:
    sys.path.insert(0, "/opt/trn_rl_repo")

VOCAB = 131072
D = 2048
NCORES = 8
TOK_PER_CORE = 2048
QUARTER = 32768

_TRACE = [False]
_LAST_RESULTS = [None]


def _hadamard(n):
    H = np.array([[1.0]], dtype=np.float64)
    while H.shape[0] < n:
        H = np.block([[H, H], [H, -H]])
    return H


def _host_prep_weights(Qidxs, Qidxs2, codebook, codebook2, SV, Wscale, inv_resid_scale):
    H8 = _hadamard(8).astype(np.float32)
    H128 = _hadamard(128).astype(np.float32)
    ws = float(np.asarray(Wscale))
    irs = float(np.asarray(inv_resid_scale))
    cb1H = (codebook.astype(np.float32) @ H8) * ws
    cb2H = (codebook2.astype(np.float32) @ H8) * (ws * irs)

    # column permutation: dst col (2j+r)*128+i_lo takes src (i, j), i=r*128+i_lo
    i_idx = np.arange(256)
    j_idx = np.arange(8)
    dst_col = (2 * j_idx[None, :] + (i_idx >> 7)[:, None]) * 128 + (i_idx & 127)[:, None]
    perm = np.empty(D, dtype=np.int64)
    perm[dst_col.reshape(-1)] = np.arange(D)

    E = np.empty((VOCAB, D), dtype=np.float16)
    CH = 8192
    for v0 in range(0, VOCAB, CH):
        q1 = Qidxs[v0:v0 + CH].astype(np.int32) & 0xFFFF
        q2 = Qidxs2[v0:v0 + CH].astype(np.int32) & 0xFFFF
        G = (cb1H[q1] + cb2H[q2]).reshape(-1, D)
        E[v0:v0 + CH] = G[:, perm].astype(np.float16)

    SVf = SV.astype(np.float32).reshape(2, 128, 8)  # [q, p', j]
    Smv = np.empty((128, 16, 256), dtype=np.float16)  # [k, j*2+r, q*128+p']
    for j in range(8):
        for r in range(2):
            cols = np.empty((128, 256), np.float32)
            for q in range(2):
                sign = -1.0 if (q == 1 and r == 1) else 1.0
                cols[:, q * 128:(q + 1) * 128] = H128 * (sign * SVf[q, :, j])[None, :]
            Smv[:, j * 2 + r, :] = cols.astype(np.float16)
    return E, Smv


def _host_prep_tokens(flat_ids):
    """Per-core quarter-sorted slot bookkeeping. Returns caps plus per-core
    (idx_wrapped, offs, ) arrays."""
    counts = np.zeros((NCORES, 4), dtype=np.int64)
    percore = []
    for c in range(NCORES):
        v = flat_ids[c * TOK_PER_CORE:(c + 1) * TOK_PER_CORE]
        percore.append(v)
        for k in range(4):
            counts[c, k] = int(((v >> 15) == k).sum())
    caps = []
    for k in range(4):
        m = int(counts[:, k].max())
        caps.append(0 if m == 0 else int(np.ceil(m / 128) * 128))
    S = sum(caps)
    nch = S // 128

    idx_all = np.zeros((NCORES, 128, S // 16), dtype=np.int16)
    offs_all = np.full((NCORES, 128, nch), D, dtype=np.int32)  # default -> dummy row D(=2048)
    for c in range(NCORES):
        v = percore[c]
        order = np.argsort(v >> 15, kind="stable")
        col = 0
        slot_rows = np.empty(S, dtype=np.int32)
        slot_orig = np.full(S, -1, dtype=np.int32)
        for k in range(4):
            if caps[k] == 0:
                continue
            sel = order[(v[order] >> 15) == k]
            nk = len(sel)
            rows = np.full(caps[k], k * QUARTER, dtype=np.int32)
            rows[:nk] = v[sel]
            slot_rows[col:col + caps[k]] = rows
            slot_orig[col:col + nk] = sel
            col += caps[k]
        # wrapped idx arrays per 128-slot chunk, replicated across the 8 groups
        # of 16 partitions; chunk ch covers slots [ch*128, ch*128+128), all within
        # one region by construction (caps are multiples of 128).
        region_of = np.repeat(np.arange(4), [caps[k] for k in range(4)])
        for ch in range(nch):
            sl = slot_rows[ch * 128:(ch + 1) * 128]
            k = int(region_of[ch * 128])
            local = (sl - k * QUARTER).astype(np.int16)
            wr = local.reshape(8, 16).T  # [16, 8]
            idx_all[c, :, ch * 8:(ch + 1) * 8] = np.tile(wr, (8, 1))
        for ch in range(nch):
            so = slot_orig[ch * 128:(ch + 1) * 128]
            offs_all[c, :, ch] = np.where(so >= 0, so, D)
    return caps, S, nch, idx_all, offs_all


def _build_program(caps, S, nch):
    import concourse.bacc as bacc
    import concourse.bass as bass
    import concourse.mybir as mybir
    from concourse.bass import IndirectOffsetOnAxis
    from concourse.library_config import mlp
    from contextlib import ExitStack

    regions = [k for k in range(4) if caps[k] > 0]
    # gather calls: per region, groups of up to 512 slots (multiples of 128);
    # chunk ch -> (call index, local t0 within call); call -> (region, slot0, size)
    calls = []
    chunk_map = []
    slot0 = 0
    for k in regions:
        done = 0
        while done < caps[k]:
            csz = min(512, caps[k] - done)
            ci = len(calls)
            calls.append((k, slot0 + done, csz))
            for t0 in range(0, csz, 128):
                chunk_map.append((ci, t0))
            done += csz
        slot0 += caps[k]
    assert len(chunk_map) == nch

    nc = bacc.Bacc("TRN2", debug=True, num_swdge_queues=4)
    E_d = nc.dram_tensor("E", [VOCAB, D], mybir.dt.float16, kind="ExternalInput")
    S_d = nc.dram_tensor("Smv", [128, 16 * 256], mybir.dt.float16, kind="ExternalInput")
    idx_d = nc.dram_tensor("idx", [128, S // 16], mybir.dt.int16, kind="ExternalInput")
    offs_d = nc.dram_tensor("offs", [128, nch], mybir.dt.int32, kind="ExternalInput")
    out_d = nc.dram_tensor("out", [D + 1, D], mybir.dt.float16, kind="ExternalOutput")

    with (
        nc.Block() as block,
        ExitStack() as st,
        nc.semaphore("io") as io,
        nc.semaphore("gsem0") as gsem0,
        nc.semaphore("gsem1") as gsem1,
        nc.semaphore("gsem2") as gsem2,
        nc.semaphore("gsem3") as gsem3,
        nc.semaphore("mmsem") as mmsem,
        nc.semaphore("evsemA") as evsemA,
        nc.semaphore("evsemB") as evsemB,
        nc.semaphore("scsem") as scsem,
    ):
        smv_sb = st.enter_context(nc.sbuf_tensor("smv", [128, 16, 256], mybir.dt.float16))
        idx_sb = st.enter_context(nc.sbuf_tensor("idxs", [128, S // 16], mybir.dt.int16))
        x_c = [st.enter_context(nc.sbuf_tensor(f"x{ci}", [128, 16, csz], mybir.dt.float16))
               for ci, (k, s0, csz) in enumerate(calls)]
        T2N = 8
        t2 = [st.enter_context(nc.sbuf_tensor(f"t2_{i}", [128, D], mybir.dt.float16))
              for i in range(T2N)]
        fo_sb = st.enter_context(nc.sbuf_tensor("fo", [128, nch], mybir.dt.int32))
        ps = [st.enter_context(nc.psum_tensor(f"ps{i}", [128, 8, 2, 128], mybir.dt.float32))
              for i in range(2)]

        @block.sync
        def _(sync):
            sync.dma_start(idx_sb[:, :], idx_d[:, :]).then_inc(io, 16)
            sync.dma_start(smv_sb[:, :, :], S_d[:, :].rearrange("p (a b) -> p a b", a=16)).then_inc(io, 16)
            sync.dma_start(fo_sb[:, :], offs_d[:, :]).then_inc(io, 16)

        gsems = [gsem0, gsem1, gsem2, gsem3]

        def emit_gather(gpsimd, ci):
            k, s0, csz = calls[ci]
            gpsimd.dma_gather(
                x_c[ci][:, :, :],
                E_d[k * QUARTER:(k + 1) * QUARTER, :],
                idx_sb[:, s0 // 16:(s0 + csz) // 16],
                csz, csz, D,
                transpose=True,
                queue_num=ci % 4,
            ).then_inc(gsems[ci % 4], 16)

        @block.gpsimd
        def _(gpsimd):
            gpsimd.load_library(mlp)
            gpsimd.wait_ge(io, 16)
            for ci in range(len(calls)):
                emit_gather(gpsimd, ci)
            gpsimd.wait_ge(io, 48)
            for ch in range(nch):
                if ch % 2 == 0:
                    gpsimd.wait_ge(evsemA, ch // 2 + 1)
                else:
                    gpsimd.wait_ge(evsemB, (ch + 1) // 2)
                gpsimd.indirect_dma_start(
                    out_d[:, :],
                    IndirectOffsetOnAxis(ap=fo_sb[:, ch:ch + 1], axis=0),
                    t2[ch % T2N][:, :],
                    None,
                ).then_inc(scsem, 16)

        @block.tensor
        def _(tensor):
            import concourse.bass as bass
            tensor.wait_ge(io, 32)
            for ch, (ci, t0) in enumerate(chunk_map):
                tensor.wait_ge(gsems[ci % 4], 16 * (ci // 4 + 1))
                if ch >= 2:
                    if ch % 2 == 0:
                        tensor.wait_ge(evsemA, ch // 2)
                    else:
                        tensor.wait_ge(evsemB, (ch - 1) // 2)
                p = ps[ch % 2]
                for j in range(8):
                    for r in range(2):
                        mm = tensor.matmul(
                            p[:, j, :, :],
                            bass.AP(x_c[ci], (2 * j + r) * calls[ci][2] + t0,
                                    [[16 * calls[ci][2], 128], [1, 128]]),
                            smv_sb[:, j * 2 + r, :],
                            start=(r == 0),
                            stop=(r == 1),
                        )
                        if j == 7 and r == 1:
                            mm.then_inc(mmsem)

        # evac enumeration (q, p', j): contiguous 8-elem writes, strided reads
        # (strided *writes* cost ~4.4ns/elem; strided reads are free).
        def evac_aps(ch):
            p = ps[ch % 2]
            out_ap = bass.AP(t2[ch % T2N], 0, [[D, 128], [1024, 2], [8, 128], [1, 8]])
            in_ap = bass.AP(p, 0, [[2048, 128], [128, 2], [1, 128], [256, 8]])
            return out_ap, in_ap

        @block.scalar
        def _(scalar):
            import concourse.mybir as mybir
            for ch in range(0, nch, 2):
                scalar.wait_ge(mmsem, ch + 1)
                if ch >= T2N:
                    scalar.wait_ge(scsem, 16 * (ch - T2N + 1))
                out_ap, in_ap = evac_aps(ch)
                scalar.activation(out_ap, in_ap, mybir.ActivationFunctionType.Copy).then_inc(evsemA)

        @block.vector
        def _(vector):
            for ch in range(1, nch, 2):
                vector.wait_ge(mmsem, ch + 1)
                if ch >= T2N:
                    vector.wait_ge(scsem, 16 * (ch - T2N + 1))
                out_ap, in_ap = evac_aps(ch)
                vector.tensor_copy(out_ap, in_ap).then_inc(evsemB)

    nc.compile()
    return nc


def kernel(input_ids, Qidxs, Qidxs2, codebook, codebook2, SV, Wscale, inv_resid_scale):
    from concourse.bass_utils import run_bass_kernel_spmd

    input_ids = np.asarray(input_ids)
    E, Smv = _host_prep_weights(np.asarray(Qidxs), np.asarray(Qidxs2),
                                np.asarray(codebook), np.asarray(codebook2),
                                np.asarray(SV), Wscale, inv_resid_scale)
    flat = input_ids.reshape(-1).astype(np.int64)
    caps, S, nch, idx_all, offs_all = _host_prep_tokens(flat)
    nc = _build_program(caps, S, nch)

    smv_flat = Smv.reshape(128, 16 * 256)
    in_maps = []
    for c in range(NCORES):
        in_maps.append({
            "E": E,
            "Smv": smv_flat,
            "idx": idx_all[c],
            "offs": offs_all[c],
        })
    res = run_bass_kernel_spmd(nc, in_maps, core_ids=list(range(NCORES)),
                               trace=_TRACE[0])
    _LAST_RESULTS[0] = res
    out = np.empty((NCORES, TOK_PER_CORE, D), dtype=np.float16)
    for c in range(NCORES):
        out[c] = res.results[c]["out"][:D]
    return out.reshape(input_ids.shape + (D,))


# revision 12
# speedup vs baseline: 1.4596x; 1.0466x over previous
"""E8 RHT Embedding kernel for Trainium2 (8 NeuronCores, data-parallel over tokens).

Math (reference): out[t] = SV * H2048( Wscale * (cb1[Qidxs[id_t]] + irs*cb2[Qidxs2[id_t]]) )
with H2048 the unnormalized Sylvester Hadamard transform over 2048 dims,
each Qidxs row holding 256 uint16 codebook indices of 8-dim E8 entries.

Strategy:
  Host (weight-only preprocessing, no token-dependent compute):
    - Fold H8 (inner Kronecker factor) + Wscale/inv_resid_scale into the codebooks,
      pre-dequantize the whole embedding table E[v] (131072 x 2048 fp16), and permute
      columns so the on-chip transposing gather lands data matmul-ready:
        E[v, (2j+r)*128 + i_lo] = (cb1H[Qidxs[v,i]] + cb2H[Qidxs2[v,i]])[j],  i = r*128+i_lo
    - Fold H2 (outer sign) and the SV output scale into 16 moving matrices
      Smov[j,r][k, q*128+p'] = sign(q,r) * SV[(q*128+p')*8+j] * H128[k, p'].
  Device (per core, 2048 tokens):
    - gpsimd dma_gather(transpose=True): fetch token rows of E (4KB each) transposed
      into SBUF X[i_lo, 2j+r, slot]. int16 gather indices only span 32768 rows, so
      tokens are processed sorted by vocab quarter (4 gather calls), padded to 128.
    - TensorE: per 128-slot chunk, per j: PSUM[t, (q,p')] = X_chunk^T-contraction with
      Smov (gathered data is the *stationary* operand), accumulating the two r halves.
      This directly yields token-major output rows - no transpose needed.
    - ScalarE: PSUM (j,q,p') fp32 -> SBUF token rows (q,p',j) fp16.
    - gpsimd indirect scatter: write each token's 4KB row to its original position
      (padding slots go to a dummy trailing row).
"""
import sys
import numpy as np

if "/opt/trn_rl_repo" not in sys.path:
    sys.path.insert(0, "/opt/trn_rl_repo")

VOCAB = 131072
D = 2048
NCORES = 8
TOK_PER_CORE = 2048
QUARTER = 32768

_TRACE = [False]
_LAST_RESULTS = [None]


def _hadamard(n):
    H = np.array([[1.0]], dtype=np.float64)
    while H.shape[0] < n:
        H = np.block([[H, H], [H, -H]])
    return H


def _host_prep_weights(Qidxs, Qidxs2, codebook, codebook2, SV, Wscale, inv_resid_scale):
    H8 = _hadamard(8).astype(np.float32)
    H128 = _hadamard(128).astype(np.float32)
    ws = float(np.asarray(Wscale))
    irs = float(np.asarray(inv_resid_scale))
    cb1H = (codebook.astype(np.float32) @ H8) * ws
    cb2H = (codebook2.astype(np.float32) @ H8) * (ws * irs)

    # column permutation: dst col (2j+r)*128+i_lo takes src (i, j), i=r*128+i_lo
    i_idx = np.arange(256)
    j_idx = np.arange(8)
    dst_col = (2 * j_idx[None, :] + (i_idx >> 7)[:, None]) * 128 + (i_idx & 127)[:, None]
    perm = np.empty(D, dtype=np.int64)
    perm[dst_col.reshape(-1)] = np.arange(D)

    E = np.empty((VOCAB, D), dtype=np.float16)
    CH = 8192
    for v0 in range(0, VOCAB, CH):
        q1 = Qidxs[v0:v0 + CH].astype(np.int32) & 0xFFFF
        q2 = Qidxs2[v0:v0 + CH].astype(np.int32) & 0xFFFF
        G = (cb1H[q1] + cb2H[q2]).reshape(-1, D)
        E[v0:v0 + CH] = G[:, perm].astype(np.float16)

    SVf = SV.astype(np.float32).reshape(2, 128, 8)  # [q, p', j]
    Smv = np.empty((128, 16, 256), dtype=np.float16)  # [k, j*2+r, q*128+p']
    for j in range(8):
        for r in range(2):
            cols = np.empty((128, 256), np.float32)
            for q in range(2):
                sign = -1.0 if (q == 1 and r == 1) else 1.0
                cols[:, q * 128:(q + 1) * 128] = H128 * (sign * SVf[q, :, j])[None, :]
            Smv[:, j * 2 + r, :] = cols.astype(np.float16)
    return E, Smv


def _host_prep_tokens(flat_ids):
    """Per-core quarter-sorted slot bookkeeping. Returns caps plus per-core
    (idx_wrapped, offs, ) arrays."""
    counts = np.zeros((NCORES, 4), dtype=np.int64)
    percore = []
    for c in range(NCORES):
        v = flat_ids[c * TOK_PER_CORE:(c + 1) * TOK_PER_CORE]
        percore.append(v)
        for k in range(4):
            counts[c, k] = int(((v >> 15) == k).sum())
    caps = []
    for k in range(4):
        m = int(counts[:, k].max())
        caps.append(0 if m == 0 else int(np.ceil(m / 128) * 128))
    S = sum(caps)
    nch = S // 128

    idx_all = np.zeros((NCORES, 128, S // 16), dtype=np.int16)
    offs_all = np.full((NCORES, 128, nch), D, dtype=np.int32)  # default -> dummy row D(=2048)
    for c in range(NCORES):
        v = percore[c]
        order = np.argsort(v >> 15, kind="stable")
        col = 0
        slot_rows = np.empty(S, dtype=np.int32)
        slot_orig = np.full(S, -1, dtype=np.int32)
        for k in range(4):
            if caps[k] == 0:
                continue
            sel = order[(v[order] >> 15) == k]
            nk = len(sel)
            rows = np.full(caps[k], k * QUARTER, dtype=np.int32)
            rows[:nk] = v[sel]
            slot_rows[col:col + caps[k]] = rows
            slot_orig[col:col + nk] = sel
            col += caps[k]
        # wrapped idx arrays per 128-slot chunk, replicated across the 8 groups
        # of 16 partitions; chunk ch covers slots [ch*128, ch*128+128), all within
        # one region by construction (caps are multiples of 128).
        region_of = np.repeat(np.arange(4), [caps[k] for k in range(4)])
        for ch in range(nch):
            sl = slot_rows[ch * 128:(ch + 1) * 128]
            k = int(region_of[ch * 128])
            local = (sl - k * QUARTER).astype(np.int16)
            wr = local.reshape(8, 16).T  # [16, 8]
            idx_all[c, :, ch * 8:(ch + 1) * 8] = np.tile(wr, (8, 1))
        for ch in range(nch):
            so = slot_orig[ch * 128:(ch + 1) * 128]
            offs_all[c, :, ch] = np.where(so >= 0, so, D)
    return caps, S, nch, idx_all, offs_all


def _build_program(caps, S, nch):
    import concourse.bacc as bacc
    import concourse.bass as bass
    import concourse.mybir as mybir
    from concourse.bass import IndirectOffsetOnAxis
    from concourse.library_config import mlp
    from contextlib import ExitStack

    regions = [k for k in range(4) if caps[k] > 0]
    # gather calls: per region, groups of up to 512 slots (multiples of 128);
    # chunk ch -> (call index, local t0 within call); call -> (region, slot0, size)
    calls = []
    chunk_map = []
    slot0 = 0
    first = True
    for k in regions:
        done = 0
        while done < caps[k]:
            csz = min(128 if first else 512, caps[k] - done)
            first = False
            ci = len(calls)
            calls.append((k, slot0 + done, csz))
            for t0 in range(0, csz, 128):
                chunk_map.append((ci, t0))
            done += csz
        slot0 += caps[k]
    assert len(chunk_map) == nch

    nc = bacc.Bacc("TRN2", debug=True, num_swdge_queues=4)
    E_d = nc.dram_tensor("E", [VOCAB, D], mybir.dt.float16, kind="ExternalInput")
    S_d = nc.dram_tensor("Smv", [128, 16 * 256], mybir.dt.float16, kind="ExternalInput")
    idx_d = nc.dram_tensor("idx", [128, S // 16], mybir.dt.int16, kind="ExternalInput")
    offs_d = nc.dram_tensor("offs", [128, nch], mybir.dt.int32, kind="ExternalInput")
    out_d = nc.dram_tensor("out", [D + 1, D], mybir.dt.float16, kind="ExternalOutput")

    with (
        nc.Block() as block,
        ExitStack() as st,
        nc.semaphore("io") as io,
        nc.semaphore("gsem0") as gsem0,
        nc.semaphore("gsem1") as gsem1,
        nc.semaphore("gsem2") as gsem2,
        nc.semaphore("gsem3") as gsem3,
        nc.semaphore("mmsem") as mmsem,
        nc.semaphore("evsemA") as evsemA,
        nc.semaphore("evsemB") as evsemB,
        nc.semaphore("scsem") as scsem,
    ):
        smv_sb = st.enter_context(nc.sbuf_tensor("smv", [128, 16, 256], mybir.dt.float16))
        idx_sb = st.enter_context(nc.sbuf_tensor("idxs", [128, S // 16], mybir.dt.int16))
        x_c = [st.enter_context(nc.sbuf_tensor(f"x{ci}", [128, 16, csz], mybir.dt.float16))
               for ci, (k, s0, csz) in enumerate(calls)]
        T2N = 8
        t2 = [st.enter_context(nc.sbuf_tensor(f"t2_{i}", [128, D], mybir.dt.float16))
              for i in range(T2N)]
        fo_sb = st.enter_context(nc.sbuf_tensor("fo", [128, nch], mybir.dt.int32))
        ps = [st.enter_context(nc.psum_tensor(f"ps{i}", [128, 8, 2, 128], mybir.dt.float32))
              for i in range(2)]

        @block.sync
        def _(sync):
            sync.dma_start(idx_sb[:, :], idx_d[:, :]).then_inc(io, 16)
            sync.dma_start(smv_sb[:, :, :], S_d[:, :].rearrange("p (a b) -> p a b", a=16)).then_inc(io, 16)
            sync.dma_start(fo_sb[:, :], offs_d[:, :]).then_inc(io, 16)

        gsems = [gsem0, gsem1, gsem2, gsem3]

        def emit_gather(gpsimd, ci):
            k, s0, csz = calls[ci]
            gpsimd.dma_gather(
                x_c[ci][:, :, :],
                E_d[k * QUARTER:(k + 1) * QUARTER, :],
                idx_sb[:, s0 // 16:(s0 + csz) // 16],
                csz, csz, D,
                transpose=True,
                queue_num=ci % 4,
            ).then_inc(gsems[ci % 4], 16)

        @block.gpsimd
        def _(gpsimd):
            gpsimd.load_library(mlp)
            gpsimd.wait_ge(io, 16)
            for ci in range(len(calls)):
                emit_gather(gpsimd, ci)
            gpsimd.wait_ge(io, 48)
            for ch in range(nch):
                gpsimd.wait_ge(evsemA, ch + 1)
                gpsimd.wait_ge(evsemB, ch + 1)
                gpsimd.indirect_dma_start(
                    out_d[:, :],
                    IndirectOffsetOnAxis(ap=fo_sb[:, ch:ch + 1], axis=0),
                    t2[ch % T2N][:, :],
                    None,
                ).then_inc(scsem, 16)

        @block.tensor
        def _(tensor):
            import concourse.bass as bass
            tensor.wait_ge(io, 32)
            for ch, (ci, t0) in enumerate(chunk_map):
                tensor.wait_ge(gsems[ci % 4], 16 * (ci // 4 + 1))
                if ch >= 2:
                    tensor.wait_ge(evsemA, ch - 1)
                    tensor.wait_ge(evsemB, ch - 1)
                p = ps[ch % 2]
                for j in range(8):
                    for r in range(2):
                        mm = tensor.matmul(
                            p[:, j, :, :],
                            bass.AP(x_c[ci], (2 * j + r) * calls[ci][2] + t0,
                                    [[16 * calls[ci][2], 128], [1, 128]]),
                            smv_sb[:, j * 2 + r, :],
                            start=(r == 0),
                            stop=(r == 1),
                        )
                        if j == 7 and r == 1:
                            mm.then_inc(mmsem)

        # evac split by j-halves so ACT reads PSUM banks 0-1 (j 0-3) while DVE
        # reads banks 2-3 (j 4-7) -- PSUM banks are single-port, concurrent
        # access to the same bank (even two readers) is a hardware fault.
        # enumeration (q, p', j-half): 8-byte contiguous write runs, strided reads.
        def evac_aps(ch, jh):
            p = ps[ch % 2]
            out_ap = bass.AP(t2[ch % T2N], jh * 4, [[D, 128], [1024, 2], [8, 128], [1, 4]])
            in_ap = bass.AP(p, jh * 4 * 256, [[2048, 128], [128, 2], [1, 128], [256, 4]])
            return out_ap, in_ap

        @block.scalar
        def _(scalar):
            import concourse.mybir as mybir
            for ch in range(nch):
                scalar.wait_ge(mmsem, ch + 1)
                if ch >= T2N:
                    scalar.wait_ge(scsem, 16 * (ch - T2N + 1))
                out_ap, in_ap = evac_aps(ch, 0)
                scalar.activation(out_ap, in_ap, mybir.ActivationFunctionType.Copy).then_inc(evsemA)

        @block.vector
        def _(vector):
            for ch in range(nch):
                vector.wait_ge(mmsem, ch + 1)
                if ch >= T2N:
                    vector.wait_ge(scsem, 16 * (ch - T2N + 1))
                out_ap, in_ap = evac_aps(ch, 1)
                vector.tensor_copy(out_ap, in_ap).then_inc(evsemB)

    nc.compile()
    return nc


def kernel(input_ids, Qidxs, Qidxs2, codebook, codebook2, SV, Wscale, inv_resid_scale):
    from concourse.bass_utils import run_bass_kernel_spmd

    input_ids = np.asarray(input_ids)
    E, Smv = _host_prep_weights(np.asarray(Qidxs), np.asarray(Qidxs2),
                                np.asarray(codebook), np.asarray(codebook2),
                                np.asarray(SV), Wscale, inv_resid_scale)
    flat = input_ids.reshape(-1).astype(np.int64)
    caps, S, nch, idx_all, offs_all = _host_prep_tokens(flat)
    nc = _build_program(caps, S, nch)

    smv_flat = Smv.reshape(128, 16 * 256)
    in_maps = []
    for c in range(NCORES):
        in_maps.append({
            "E": E,
            "Smv": smv_flat,
            "idx": idx_all[c],
            "offs": offs_all[c],
        })
    res = run_bass_kernel_spmd(nc, in_maps, core_ids=list(range(NCORES)),
                               trace=_TRACE[0])
    _LAST_RESULTS[0] = res
    out = np.empty((NCORES, TOK_PER_CORE, D), dtype=np.float16)
    for c in range(NCORES):
        out[c] = res.results[c]["out"][:D]
    return out.reshape(input_ids.shape + (D,))


# revision 13
# speedup vs baseline: 1.4873x; 1.0190x over previous
"""E8 RHT Embedding kernel for Trainium2 (8 NeuronCores, data-parallel over tokens).

Math (reference): out[t] = SV * H2048( Wscale * (cb1[Qidxs[id_t]] + irs*cb2[Qidxs2[id_t]]) )
with H2048 the unnormalized Sylvester Hadamard transform over 2048 dims,
each Qidxs row holding 256 uint16 codebook indices of 8-dim E8 entries.

Strategy:
  Host (weight-only preprocessing, no token-dependent compute):
    - Fold H8 (inner Kronecker factor) + Wscale/inv_resid_scale into the codebooks,
      pre-dequantize the whole embedding table E[v] (131072 x 2048 fp16), and permute
      columns so the on-chip transposing gather lands data matmul-ready:
        E[v, (2j+r)*128 + i_lo] = (cb1H[Qidxs[v,i]] + cb2H[Qidxs2[v,i]])[j],  i = r*128+i_lo
    - Fold H2 (outer sign) and the SV output scale into 16 moving matrices
      Smov[j,r][k, q*128+p'] = sign(q,r) * SV[(q*128+p')*8+j] * H128[k, p'].
  Device (per core, 2048 tokens):
    - gpsimd dma_gather(transpose=True): fetch token rows of E (4KB each) transposed
      into SBUF X[i_lo, 2j+r, slot]. int16 gather indices only span 32768 rows, so
      tokens are processed sorted by vocab quarter (4 gather calls), padded to 128.
    - TensorE: per 128-slot chunk, per j: PSUM[t, (q,p')] = X_chunk^T-contraction with
      Smov (gathered data is the *stationary* operand), accumulating the two r halves.
      This directly yields token-major output rows - no transpose needed.
    - ScalarE: PSUM (j,q,p') fp32 -> SBUF token rows (q,p',j) fp16.
    - gpsimd indirect scatter: write each token's 4KB row to its original position
      (padding slots go to a dummy trailing row).
"""
import sys
import numpy as np

if "/opt/trn_rl_repo" not in sys.path:
    sys.path.insert(0, "/opt/trn_rl_repo")

VOCAB = 131072
D = 2048
NCORES = 8
TOK_PER_CORE = 2048
QUARTER = 32768

_TRACE = [False]
_LAST_RESULTS = [None]


def _hadamard(n):
    H = np.array([[1.0]], dtype=np.float64)
    while H.shape[0] < n:
        H = np.block([[H, H], [H, -H]])
    return H


def _host_prep_weights(Qidxs, Qidxs2, codebook, codebook2, SV, Wscale, inv_resid_scale):
    H8 = _hadamard(8).astype(np.float32)
    H128 = _hadamard(128).astype(np.float32)
    ws = float(np.asarray(Wscale))
    irs = float(np.asarray(inv_resid_scale))
    cb1H = (codebook.astype(np.float32) @ H8) * ws
    cb2H = (codebook2.astype(np.float32) @ H8) * (ws * irs)

    # column permutation: dst col (2j+r)*128+i_lo takes src (i, j), i=r*128+i_lo
    i_idx = np.arange(256)
    j_idx = np.arange(8)
    dst_col = (2 * j_idx[None, :] + (i_idx >> 7)[:, None]) * 128 + (i_idx & 127)[:, None]
    perm = np.empty(D, dtype=np.int64)
    perm[dst_col.reshape(-1)] = np.arange(D)

    E = np.empty((VOCAB, D), dtype=np.float16)
    CH = 8192
    for v0 in range(0, VOCAB, CH):
        q1 = Qidxs[v0:v0 + CH].astype(np.int32) & 0xFFFF
        q2 = Qidxs2[v0:v0 + CH].astype(np.int32) & 0xFFFF
        G = (cb1H[q1] + cb2H[q2]).reshape(-1, D)
        E[v0:v0 + CH] = G[:, perm].astype(np.float16)

    SVf = SV.astype(np.float32).reshape(2, 128, 8)  # [q, p', j]
    Smv = np.empty((128, 16, 256), dtype=np.float16)  # [k, j*2+r, q*128+p']
    for j in range(8):
        for r in range(2):
            cols = np.empty((128, 256), np.float32)
            for q in range(2):
                sign = -1.0 if (q == 1 and r == 1) else 1.0
                cols[:, q * 128:(q + 1) * 128] = H128 * (sign * SVf[q, :, j])[None, :]
            Smv[:, j * 2 + r, :] = cols.astype(np.float16)
    return E, Smv


def _host_prep_tokens(flat_ids):
    """Per-core quarter-sorted slot bookkeeping. Returns caps plus per-core
    (idx_wrapped, offs, ) arrays."""
    counts = np.zeros((NCORES, 4), dtype=np.int64)
    percore = []
    for c in range(NCORES):
        v = flat_ids[c * TOK_PER_CORE:(c + 1) * TOK_PER_CORE]
        percore.append(v)
        for k in range(4):
            counts[c, k] = int(((v >> 15) == k).sum())
    caps = []
    for k in range(4):
        m = int(counts[:, k].max())
        caps.append(0 if m == 0 else int(np.ceil(m / 128) * 128))
    S = sum(caps)
    nch = S // 128

    idx_all = np.zeros((NCORES, 128, S // 16), dtype=np.int16)
    offs_all = np.full((NCORES, 128, nch), D, dtype=np.int32)  # default -> dummy row D(=2048)
    for c in range(NCORES):
        v = percore[c]
        order = np.argsort(v >> 15, kind="stable")
        col = 0
        slot_rows = np.empty(S, dtype=np.int32)
        slot_orig = np.full(S, -1, dtype=np.int32)
        for k in range(4):
            if caps[k] == 0:
                continue
            sel = order[(v[order] >> 15) == k]
            nk = len(sel)
            rows = np.full(caps[k], k * QUARTER, dtype=np.int32)
            rows[:nk] = v[sel]
            slot_rows[col:col + caps[k]] = rows
            slot_orig[col:col + nk] = sel
            col += caps[k]
        # wrapped idx arrays per 128-slot chunk, replicated across the 8 groups
        # of 16 partitions; chunk ch covers slots [ch*128, ch*128+128), all within
        # one region by construction (caps are multiples of 128).
        region_of = np.repeat(np.arange(4), [caps[k] for k in range(4)])
        for ch in range(nch):
            sl = slot_rows[ch * 128:(ch + 1) * 128]
            k = int(region_of[ch * 128])
            local = (sl - k * QUARTER).astype(np.int16)
            wr = local.reshape(8, 16).T  # [16, 8]
            idx_all[c, :, ch * 8:(ch + 1) * 8] = np.tile(wr, (8, 1))
        for ch in range(nch):
            so = slot_orig[ch * 128:(ch + 1) * 128]
            offs_all[c, :, ch] = np.where(so >= 0, so, D)
    return caps, S, nch, idx_all, offs_all


def _build_program(caps, S, nch):
    import concourse.bacc as bacc
    import concourse.bass as bass
    import concourse.mybir as mybir
    from concourse.bass import IndirectOffsetOnAxis
    from concourse.library_config import mlp
    from contextlib import ExitStack

    regions = [k for k in range(4) if caps[k] > 0]
    # gather calls: per region, groups of up to 512 slots (multiples of 128);
    # chunk ch -> (call index, local t0 within call); call -> (region, slot0, size)
    calls = []
    chunk_map = []
    slot0 = 0
    first = True
    for k in regions:
        done = 0
        while done < caps[k]:
            csz = min(128 if first else 512, caps[k] - done)
            first = False
            ci = len(calls)
            calls.append((k, slot0 + done, csz))
            for t0 in range(0, csz, 128):
                chunk_map.append((ci, t0))
            done += csz
        slot0 += caps[k]
    assert len(chunk_map) == nch

    nc = bacc.Bacc("TRN2", debug=True, num_swdge_queues=4)
    E_d = nc.dram_tensor("E", [VOCAB, D], mybir.dt.float16, kind="ExternalInput")
    S_d = nc.dram_tensor("Smv", [128, 16 * 256], mybir.dt.float16, kind="ExternalInput")
    idx_d = nc.dram_tensor("idx", [128, S // 16], mybir.dt.int16, kind="ExternalInput")
    offs_d = nc.dram_tensor("offs", [128, nch], mybir.dt.int32, kind="ExternalInput")
    out_d = nc.dram_tensor("out", [D + 1, D], mybir.dt.float16, kind="ExternalOutput")

    with (
        nc.Block() as block,
        ExitStack() as st,
        nc.semaphore("io") as io,
        nc.semaphore("gsem0") as gsem0,
        nc.semaphore("gsem1") as gsem1,
        nc.semaphore("gsem2") as gsem2,
        nc.semaphore("gsem3") as gsem3,
        nc.semaphore("mmsem") as mmsem,
        nc.semaphore("evsemA") as evsemA,
        nc.semaphore("evsemB") as evsemB,
        nc.semaphore("scsem") as scsem,
    ):
        smv_sb = st.enter_context(nc.sbuf_tensor("smv", [128, 16, 256], mybir.dt.float16))
        idx_sb = st.enter_context(nc.sbuf_tensor("idxs", [128, S // 16], mybir.dt.int16))
        x_c = [st.enter_context(nc.sbuf_tensor(f"x{ci}", [128, 16, csz], mybir.dt.float16))
               for ci, (k, s0, csz) in enumerate(calls)]
        T2N = 8
        t2 = [st.enter_context(nc.sbuf_tensor(f"t2_{i}", [128, D], mybir.dt.float16))
              for i in range(T2N)]
        fo_sb = st.enter_context(nc.sbuf_tensor("fo", [128, nch], mybir.dt.int32))
        ps = [st.enter_context(nc.psum_tensor(f"ps{i}", [128, 8, 2, 128], mybir.dt.float32))
              for i in range(2)]

        @block.sync
        def _(sync):
            sync.dma_start(smv_sb[:, :, :], S_d[:, :].rearrange("p (a b) -> p a b", a=16)).then_inc(io, 16)
            sync.dma_start(fo_sb[:, :], offs_d[:, :]).then_inc(io, 16)

        gsems = [gsem0, gsem1, gsem2, gsem3]

        def emit_gather(gpsimd, ci):
            k, s0, csz = calls[ci]
            gpsimd.dma_gather(
                x_c[ci][:, :, :],
                E_d[k * QUARTER:(k + 1) * QUARTER, :],
                idx_sb[:, s0 // 16:(s0 + csz) // 16],
                csz, csz, D,
                transpose=True,
                queue_num=ci % 4,
            ).then_inc(gsems[ci % 4], 16)

        def emit_scatter(gpsimd, ch):
            gpsimd.wait_ge(evsemA, ch + 1)
            gpsimd.wait_ge(evsemB, ch + 1)
            gpsimd.indirect_dma_start(
                out_d[:, :],
                IndirectOffsetOnAxis(ap=fo_sb[:, ch:ch + 1], axis=0),
                t2[ch % T2N][:, :],
                None,
            ).then_inc(scsem, 16)

        @block.gpsimd
        def _(gpsimd):
            gpsimd.dma_start(idx_sb[:, :], idx_d[:, :]).then_inc(io, 16)
            gpsimd.load_library(mlp)
            gpsimd.wait_ge(io, 16)
            # first few gathers unconditionally, then interleave scatters 1:1
            ngf = min(4, len(calls))
            for ci in range(ngf):
                emit_gather(gpsimd, ci)
            gpsimd.wait_ge(io, 48)
            sc_next = 0
            for ci in range(ngf, len(calls)):
                emit_scatter(gpsimd, sc_next)
                sc_next += 1
                emit_gather(gpsimd, ci)
            for ch in range(sc_next, nch):
                emit_scatter(gpsimd, ch)

        @block.tensor
        def _(tensor):
            import concourse.bass as bass
            tensor.wait_ge(io, 32)
            for ch, (ci, t0) in enumerate(chunk_map):
                tensor.wait_ge(gsems[ci % 4], 16 * (ci // 4 + 1))
                if ch >= 2:
                    tensor.wait_ge(evsemA, ch - 1)
                    tensor.wait_ge(evsemB, ch - 1)
                p = ps[ch % 2]
                for j in range(8):
                    for r in range(2):
                        mm = tensor.matmul(
                            p[:, j, :, :],
                            bass.AP(x_c[ci], (2 * j + r) * calls[ci][2] + t0,
                                    [[16 * calls[ci][2], 128], [1, 128]]),
                            smv_sb[:, j * 2 + r, :],
                            start=(r == 0),
                            stop=(r == 1),
                        )
                        if j == 7 and r == 1:
                            mm.then_inc(mmsem)

        # evac split by j-halves so ACT reads PSUM banks 0-1 (j 0-3) while DVE
        # reads banks 2-3 (j 4-7) -- PSUM banks are single-port, concurrent
        # access to the same bank (even two readers) is a hardware fault.
        # enumeration (q, p', j-half): 8-byte contiguous write runs, strided reads.
        def evac_aps(ch, jh):
            p = ps[ch % 2]
            out_ap = bass.AP(t2[ch % T2N], jh * 4, [[D, 128], [1024, 2], [8, 128], [1, 4]])
            in_ap = bass.AP(p, jh * 4 * 256, [[2048, 128], [128, 2], [1, 128], [256, 4]])
            return out_ap, in_ap

        @block.scalar
        def _(scalar):
            import concourse.mybir as mybir
            for ch in range(nch):
                scalar.wait_ge(mmsem, ch + 1)
                if ch >= T2N:
                    scalar.wait_ge(scsem, 16 * (ch - T2N + 1))
                out_ap, in_ap = evac_aps(ch, 0)
                scalar.activation(out_ap, in_ap, mybir.ActivationFunctionType.Copy).then_inc(evsemA)

        @block.vector
        def _(vector):
            for ch in range(nch):
                vector.wait_ge(mmsem, ch + 1)
                if ch >= T2N:
                    vector.wait_ge(scsem, 16 * (ch - T2N + 1))
                out_ap, in_ap = evac_aps(ch, 1)
                vector.tensor_copy(out_ap, in_ap).then_inc(evsemB)

    nc.compile()
    return nc


def kernel(input_ids, Qidxs, Qidxs2, codebook, codebook2, SV, Wscale, inv_resid_scale):
    from concourse.bass_utils import run_bass_kernel_spmd

    input_ids = np.asarray(input_ids)
    E, Smv = _host_prep_weights(np.asarray(Qidxs), np.asarray(Qidxs2),
                                np.asarray(codebook), np.asarray(codebook2),
                                np.asarray(SV), Wscale, inv_resid_scale)
    flat = input_ids.reshape(-1).astype(np.int64)
    caps, S, nch, idx_all, offs_all = _host_prep_tokens(flat)
    nc = _build_program(caps, S, nch)

    smv_flat = Smv.reshape(128, 16 * 256)
    in_maps = []
    for c in range(NCORES):
        in_maps.append({
            "E": E,
            "Smv": smv_flat,
            "idx": idx_all[c],
            "offs": offs_all[c],
        })
    res = run_bass_kernel_spmd(nc, in_maps, core_ids=list(range(NCORES)),
                               trace=_TRACE[0])
    _LAST_RESULTS[0] = res
    out = np.empty((NCORES, TOK_PER_CORE, D), dtype=np.float16)
    for c in range(NCORES):
        out[c] = res.results[c]["out"][:D]
    return out.reshape(input_ids.shape + (D,))
